# revision 26
# baseline (speedup 1.0000x reference)
"""CIF high-res Gaussian scatter accumulator on 8 trn2 NeuronCores.

Reference (per field f, cell (j,i) of a 38x50 grid): v,x,y,_,scale =
cif_head[f,:,j,i]; val = v/16 if v>0.1 else 0; sigma = max(1, 4*scale);
stamp a circularly-truncated Gaussian of height val around (8y, 8x) into
a [300,400] heatmap (nearest pixel gets full val), accumulate, clamp at 1.
Negative scatter indices wrap python-style; indices >= bound are dropped.

Kernel strategy (fields sharded 3-per-core, zero-padded; identical SPMD
program on all 8 cores):
  Cells on partitions (p = f*38+j, 114 of 128), i on the free axis with
  pitch 52 (2 zero pad cols per block). Big tiles pack (m, i): col =
  (m+8)*52 + i for x-offsets m in [-8,7]; y-tiles pack (u, i) likewise
  for u in [-7,7]. Per-axis precompute: Dx=(m-fx)^2 fp32, nax2=s^2-Dx,
  gx=exp(inv2*Dx) fp16, vgx=val*gx, Dy fp32, gy fp16. Main loop over u:
    mask = [Dy_u <= nax2]   (Pool TT is_le, fp32 compare => exact circle)
    p    = vgx * gy_u       (DVE TT fp16 2x; bcast has packed last dim)
    C    = p * mask         (DVE TT fp16 2x)
  The (m,i) layout keeps every operand's innermost AP dim packed, so DVE
  runs these at 2x. The shift-accumulate runs on the TensorEngine: fp16
  one-hot matrices (1/16 folded in) map cell rows to psum rows m=jb*3+f;
  two manual strided rhs APs (offset 0 / 415, dims [[1,52],[52,8]]) read
  psum col 8i+mi directly from the (m,i) tile, pad cols supplying zeros.
  Per u: 2 matmuls (+2 y-wrap for u<0 via ew37/38) into PSUM bank
  (u+8)%8 / (u+308)%8; near-pixel q tiles (|u|<=4) add 2-4 more. The
  u-order finalizes one bank every ~2 iterations; each bank is evicted
  (ACT copy), x-wrap folded, clamped to 1.0 into fp16, and DMA-ed out
  while the loop continues. Host casts fp16 -> fp32.
"""

import sys

import numpy as np

if "/opt/trn_rl_repo" not in sys.path:
    sys.path.insert(0, "/opt/trn_rl_repo")

F_TOTAL, HF, WF = 17, 38, 50
HH, WW = 300, 400
NF = 3                  # fields per core (last cores padded with zeros)
NCORES = 8
P = NF * HF             # 114 cell partitions
MOUT = NF * 39          # 117 psum partitions (39 row-blocks per field)
NM = 16                 # x offsets m in [-8, 7], stored at block m+8
NU = 15                 # y offsets u in [-7, 7], stored at block u+7
PIT = 52                # column pitch per block (50 data + 2 zero pads)
XW = NM * PIT           # 832
YW = NU * PIT           # 780
BANK = 512              # fp32 elems per PSUM bank
OB = 408                # outsb block: col 0..7 wrap px<0, 8..407 data
# u processing order = slot order of all (u,i)-packed tiles. Non-near u's
# first (near-pre overlaps them), then the 9 near u's as 3 trios, and a
# cheap final u (bank 5 completes with just 2 matmuls on the tail).
U_ORDER = [-7, -6, 6, -5, 7, -4, 4, 0, -3, 1, -2, 2, -1, 3, 5]

_cache: dict = {}


def _host_consts():
    e0 = np.zeros((P, MOUT), np.float16)
    e1 = np.zeros((P, MOUT), np.float16)
    for f in range(NF):
        for j in range(HF):
            e0[f * HF + j, j * NF + f] = 1.0 / 16.0
            e1[f * HF + j, (j + 1) * NF + f] = 1.0 / 16.0
    # y-wrap matrices: cell row j=0 with u<0 lands at Y=u+300 (negative
    # indices wrap python-style in the reference scatter)
    ew37 = np.zeros((P, MOUT), np.float16)
    ew38 = np.zeros((P, MOUT), np.float16)
    for f in range(NF):
        ew37[f * HF + 0, 37 * NF + f] = 1.0 / 16.0
        ew38[f * HF + 0, 38 * NF + f] = 1.0 / 16.0
    epack = np.concatenate([e0, e1, ew37, ew38], axis=1)

    # gpack: mg (m,i) | mgu (u,i) | g8i | g8j   (fp32; all pads zero)
    gp = np.zeros((P, XW + YW + 2 * PIT), np.float32)
    mg = np.zeros((NM, PIT), np.float32)
    mg[:, :WF] = np.arange(-8, 8, dtype=np.float32)[:, None]
    gp[:, 0:XW] = mg.reshape(-1)[None, :]
    mgu = np.zeros((NU, PIT), np.float32)
    mgu[:, :WF] = np.array(U_ORDER, dtype=np.float32)[:, None]
    gp[:, XW : XW + YW] = mgu.reshape(-1)[None, :]
    g8i = np.zeros(PIT, np.float32)
    g8i[:WF] = 8.0 * np.arange(WF, dtype=np.float32)
    gp[:, XW + YW : XW + YW + PIT] = g8i[None, :]
    jj = np.tile(np.arange(HF, dtype=np.float32), NF)
    gp[:, XW + YW + PIT : XW + YW + PIT + WF] = (8.0 * jj)[:, None]
    return {"epack": epack, "gpack": gp}


def _build_program():
    import concourse.bass as bass  # noqa: F401
    import concourse.mybir as mybir
    from concourse.bacc import Bacc
    from concourse.tile import TileContext
    from bass_rust import AP as RawAP

    Alu = mybir.AluOpType
    Act = mybir.ActivationFunctionType
    f32 = mybir.dt.float32
    f16 = mybir.dt.float16

    nc = Bacc()
    cif = nc.declare_dram_parameter("cif", [NF, 5, HF, WF], f32, isOutput=False)
    ep_d = nc.declare_dram_parameter("epack", [P, 4 * MOUT], f16, isOutput=False)
    gp_d = nc.declare_dram_parameter("gpack", [P, XW + YW + 2 * PIT], f32,
                                     isOutput=False)
    out_d = nc.declare_dram_parameter("out", [NF, HH, WW], f16, isOutput=True)

    with TileContext(nc) as tc:
        with tc.tile_pool(name="sb", bufs=1) as sp, tc.tile_pool(
            name="ps", bufs=1, space="PSUM"
        ) as pp:
            # ---- constants + inputs ----
            ep_t = sp.tile([P, 4 * MOUT], f16, name="ep", tag="ep")
            gp_t = sp.tile([P, XW + YW + 2 * PIT], f32, name="gp", tag="gp")
            e0_t = ep_t[:, 0 * MOUT : 1 * MOUT]
            e1_t = ep_t[:, 1 * MOUT : 2 * MOUT]
            ew37_t = ep_t[:, 2 * MOUT : 3 * MOUT]
            ew38_t = ep_t[:, 3 * MOUT : 4 * MOUT]
            mg_v = gp_t[:, 0:XW]
            mgu_v = gp_t[:, XW : XW + YW]
            g8i_v = gp_t[:, XW + YW : XW + YW + PIT]
            g8j_v = gp_t[:, XW + YW + PIT : XW + YW + 2 * PIT]

            # input: one DMA per field loads all 5 channels side-by-side
            # (j on partitions, c*52+i on free); pads pre-zeroed
            chall = sp.tile([P, 5 * PIT], f32, name="chall", tag="chall")
            nc.gpsimd.memset(chall[:], 0.0)

            # chall split across SP/SP/Pool queues; SP also loads g8 + mg
            # (gates of the x chain); ACT queue loads mgu + epack
            def chall_dma(eng, f):
                eng.dma_start(
                    out=chall[f * HF : (f + 1) * HF, :].rearrange(
                        "p (c i) -> p c i", c=5
                    )[:, :, 0:WF],
                    in_=cif[f].transpose([1, 0, 2]),
                )

            nc.sync.dma_start(out=gp_t[:, XW + YW :], in_=gp_d[:, XW + YW :])
            chall_dma(nc.sync, 0)
            chall_dma(nc.sync, 1)
            chall_dma(nc.gpsimd, 2)
            nc.gpsimd.dma_start(out=gp_t[:, XW : XW + YW],
                                in_=gp_d[:, XW : XW + YW])
            nc.scalar.dma_start(out=gp_t[:, 0:XW], in_=gp_d[:, 0:XW])
            nc.scalar.dma_start(out=ep_t[:], in_=ep_d[:])
            ch_v = chall[:, 0 * PIT : 1 * PIT]
            ch_x = chall[:, 1 * PIT : 2 * PIT]
            ch_y = chall[:, 2 * PIT : 3 * PIT]
            ch_s = chall[:, 4 * PIT : 5 * PIT]

            # ---- per-cell smalls [P, 52] ----
            def small(tag, dt=f32):
                return sp.tile([P, PIT], dt, name=tag, tag=tag)

            val_t, fx_t, fy_t = small("val"), small("fx"), small("fy")
            sg_t, sg2_t, inv_t, inv2_t = (
                small("sg"), small("sg2"), small("inv"), small("inv2"),
            )
            valh_t = small("valh", f16)
            # fx = 8*x - 8*i ; fy = 8*y - 8*j  (first: they gate the chain)
            nc.vector.scalar_tensor_tensor(
                out=fx_t[:], in0=ch_x[:], scalar=8.0, in1=g8i_v[:],
                op0=Alu.mult, op1=Alu.subtract,
            )
            nc.vector.scalar_tensor_tensor(
                out=fy_t[:], in0=ch_y[:], scalar=8.0, in1=g8j_v[:],
                op0=Alu.mult, op1=Alu.subtract,
            )
            # sigma = max(1, 4*scale); inv2 = -0.5/sigma^2
            nc.vector.tensor_scalar(
                out=sg_t[:], in0=ch_s[:], scalar1=4.0, scalar2=1.0,
                op0=Alu.mult, op1=Alu.max,
            )
            nc.vector.tensor_tensor(
                out=sg2_t[:], in0=sg_t[:], in1=sg_t[:], op=Alu.mult
            )
            nc.vector.reciprocal(inv_t[:], sg2_t[:])
            nc.vector.tensor_scalar(
                out=inv2_t[:], in0=inv_t[:], scalar1=-0.5, scalar2=None,
                op0=Alu.mult,
            )
            # val = (v > 0.1) * v   (the 1/16 scale lives in E matrices)
            nc.vector.scalar_tensor_tensor(
                out=val_t[:], in0=ch_v[:], scalar=0.1,
                in1=ch_v[:], op0=Alu.is_gt, op1=Alu.mult,
            )
            nc.vector.tensor_copy(out=valh_t[:], in_=val_t[:])

            # ---- packed per-axis bigs ----
            def bigx(tag, dt):
                return sp.tile([P, XW], dt, name=tag, tag=tag)

            def bigy(tag, dt):
                return sp.tile([P, YW], dt, name=tag, tag=tag)

            def vx(t):  # [P, 16, 52] view
                return t[:].rearrange("p (m i) -> p m i", i=PIT)

            def vx1(t):  # [P, 15, 52] view, m-blocks 1..15 (m=-8 dropped:
                # |dx|>=4 there, always outside the truncation circle)
                return t[:, PIT:XW].rearrange("p (m i) -> p m i", i=PIT)

            def vy(t):  # [P, 15, 52] view
                return t[:].rearrange("p (u i) -> p u i", i=PIT)

            def bcx(col, reps=NM):  # [P,52] -> [P, reps, 52] bcast
                return col.unsqueeze(1).broadcast_to([P, reps, PIT])

            scr1, dxs_t = bigx("scr1", f32), bigx("dxs", f32)
            hyall = bigy("hyall", f32)      # hy_u = sg2 - Dy_u per u column
            scr2, dys_t = bigy("scr2", f32), bigy("dys", f32)
            es1, gx_t, vgx_t = bigx("es1", f16), bigx("gx", f16), bigx("vgx", f16)
            es2, gy_t = bigy("es2", f16), bigy("gy", f16)
            nx_t, nxv_t = bigx("nx", f16), bigx("nxv", f16)
            ny_t = bigy("ny", f16)

            # Dx = (mg - fx)^2 ; gx = exp(inv2*Dx)   (m-blocks 1..15 only)
            nc.gpsimd.tensor_tensor(
                out=vx1(scr1),
                in0=gp_t[:, PIT:XW].rearrange("p (m i) -> p m i", i=PIT),
                in1=bcx(fx_t[:], NU),
                op=Alu.subtract,
            )
            nc.scalar.square(dxs_t[:, PIT:], scr1[:, PIT:])
            nc.gpsimd.tensor_tensor(
                out=vx1(es1), in0=vx1(dxs_t), in1=bcx(inv2_t[:], NU),
                op=Alu.mult,
            )
            nc.scalar.activation(gx_t[:, PIT:], es1[:, PIT:], Act.Exp)
            nc.vector.tensor_tensor(
                out=vx1(vgx_t), in0=vx1(gx_t), in1=bcx(valh_t[:], NU),
                op=Alu.mult,
            )
            # Dy = (mgu - fy)^2 ; gy = exp(inv2*Dy)
            nc.gpsimd.tensor_tensor(
                out=vy(scr2), in0=vy(mgu_v), in1=bcx(fy_t[:], NU),
                op=Alu.subtract,
            )
            nc.scalar.square(dys_t[:], scr2[:])
            nc.gpsimd.tensor_tensor(
                out=vy(es2), in0=vy(dys_t), in1=bcx(inv2_t[:], NU), op=Alu.mult
            )
            nc.scalar.activation(gy_t[:], es2[:], Act.Exp)

            # ---- nearest-pixel precompute (emitted lazily at first near u) --
            dxm_t, dym_t, dn_t, gn_t, vgn_t, val2_t = (
                small("dxm"), small("dym"), small("dn"),
                small("gn"), small("vgn"), small("val2"),
            )
            val2h_t = small("val2h", f16)

            def emit_near_pre():
                nc.vector.tensor_reduce(
                    out=dxm_t[:], in_=vx1(dxs_t).transpose([0, 2, 1]),
                    axis=mybir.AxisListType.X, op=Alu.min,
                )
                nc.vector.tensor_reduce(
                    out=dym_t[:], in_=vy(dys_t).transpose([0, 2, 1]),
                    axis=mybir.AxisListType.X, op=Alu.min,
                )
                nc.vector.tensor_tensor(
                    out=dn_t[:], in0=dxm_t[:], in1=dym_t[:], op=Alu.add
                )
                nc.vector.tensor_tensor(
                    out=dn_t[:], in0=dn_t[:], in1=inv2_t[:], op=Alu.mult
                )
                nc.scalar.activation(gn_t[:], dn_t[:], Act.Exp)
                nc.vector.tensor_tensor(
                    out=vgn_t[:], in0=gn_t[:], in1=val_t[:], op=Alu.mult
                )
                nc.vector.tensor_tensor(
                    out=val2_t[:], in0=val_t[:], in1=vgn_t[:], op=Alu.subtract
                )
                nc.vector.tensor_copy(out=val2h_t[:], in_=val2_t[:])
                nc.vector.tensor_scalar(
                    out=nx_t[:, PIT:], in0=dxs_t[:, PIT:], scalar1=0.25,
                    scalar2=None, op0=Alu.is_lt,
                )
                nc.gpsimd.tensor_tensor(
                    out=vx1(nxv_t), in0=vx1(nx_t), in1=bcx(val2h_t[:], NU),
                    op=Alu.mult,
                )
                nc.vector.tensor_scalar(
                    out=ny_t[:], in0=dys_t[:], scalar1=0.25, scalar2=None,
                    op0=Alu.is_lt,
                )

            # ---- work tiles (slot-major mega-tiles; subtile deps) ----
            SLOT = {u: k for k, u in enumerate(U_ORDER)}
            s2all = sp.tile([P, NU * XW], f16, name="s2all", tag="s2all")
            mkall = sp.tile([P, NU * XW], f16, name="mkall", tag="mkall")
            pall = sp.tile([P, NU * XW], f16, name="pall", tag="pall")
            call = sp.tile([P, NU * XW], f16, name="call", tag="call")
            qq = [bigx(f"qq{k}", f16) for k in range(3)]
            for k in range(NU):
                nc.gpsimd.memset(call[:, k * XW : k * XW + PIT], 0.0)
            for t in qq:
                # near q lives in m-blocks [4,13); borders must stay zero
                nc.gpsimd.memset(t[:, 0 : 4 * PIT], 0.0)
                nc.gpsimd.memset(t[:, 13 * PIT :], 0.0)

            def tv(t, k0, n):  # [P, n, 15, 52]: slots k0..k0+n, m-blocks 1..15
                return t[:].rearrange("p (k m i) -> p k m i", m=NM, i=PIT)[
                    :, k0 : k0 + n, 1:NM, :
                ]

            acc = pp.tile([MOUT, 8 * BANK], f32, name="acc", tag="acc",
                          space="PSUM")
            outsb = sp.tile([MOUT, 8 * OB], f16, name="outsb", tag="outsb")

            # Bank schedule: primary matmul into bank (u+8)%8; for u<0 an
            # extra y-wrap matmul (row j=0 -> Y=u+300) into bank (u+308)%8.
            def bank_of(u, wrap):
                return (u + 308) % 8 if wrap else (u + 8) % 8

            sched = []  # (u, kind, wrap) -> 2 matmuls each (B and A half)
            for u in range(-7, 8):
                sched.append((u, "C", False))
                if u < 0:
                    sched.append((u, "C", True))
                if abs(u) <= 4:
                    sched.append((u, "q", False))
                    if u < 0:
                        sched.append((u, "q", True))
            bank_total = [0] * 8
            for u, kind, wrap in sched:
                bank_total[bank_of(u, wrap)] += 2
            bank_done = [0] * 8
            completion = {b: 0 for b in range(8)}
            for u, kind, wrap in sched:
                b = bank_of(u, wrap)
                completion[b] = max(completion[b], SLOT[u])

            def rhs_ap(tile, slot, half):
                base = tile[:]
                pstride = base.ap[0][0]
                off = base.offset + slot * XW + (415 if half else 0)
                return RawAP(base.tensor, off,
                             [[pstride, P], [1, PIT], [PIT, 8]])

            def mm(rhs_tile, slot, u, wrap):
                b = bank_of(u, wrap)
                if wrap:
                    lhs = ew37_t if (u + 308) // 8 == 37 else ew38_t
                else:
                    lhs = e0_t if u < 0 else e1_t
                for half in (0, 1):
                    nc.tensor.matmul(
                        out=acc[:, b * BANK : b * BANK + 416],
                        lhsT=lhs,
                        rhs=rhs_ap(rhs_tile, slot, half),
                        start=(bank_done[b] == 0),
                        stop=(bank_done[b] == bank_total[b] - 1),
                    )
                    bank_done[b] += 1

            POOL_EPI_BANKS = {4, 0, 1, 6}   # wrap+min on Pool for balance

            def bank_epilogue(b):
                blk = outsb[:, b * OB : (b + 1) * OB]
                bk = acc[:, b * BANK : b * BANK + OB]
                nc.scalar.copy(out=blk, in_=bk)
                eng = nc.gpsimd if b in POOL_EPI_BANKS else nc.vector
                # x-wrap: psum cols [0,8) hold px<0 -> add at X=px+400
                eng.tensor_tensor(
                    out=blk[:, 400:408], in0=blk[:, 400:408],
                    in1=blk[:, 0:8], op=Alu.add,
                )
                eng.tensor_scalar(
                    out=blk[:, 8:408], in0=blk[:, 8:408], scalar1=1.0,
                    scalar2=None, op0=Alu.min,
                )
                nc.sync.dma_start(
                    out=out_d[:, b : b + 8 * 36 + 1 : 8, :].transpose(
                        [1, 0, 2]
                    ),
                    in_=outsb[NF : 38 * NF, b * OB + 8 : b * OB + 408],
                )
                if b < 4:
                    nc.scalar.dma_start(
                        out=out_d[:, 296 + b, :],
                        in_=outsb[38 * NF : 39 * NF,
                                  b * OB + 8 : b * OB + 408],
                    )

            # s2 = (sg2 - Dy_u) - Dx on Pool: fp32-internal subtract, fp16
            # write keeps the sign => circle boundary matches fp32 exactly
            def emit_s2(k):
                hy = hyall[:, k * PIT : (k + 1) * PIT]
                nc.gpsimd.tensor_tensor(
                    out=hy, in0=sg2_t[:],
                    in1=dys_t[:, k * PIT : (k + 1) * PIT], op=Alu.subtract,
                )
                nc.gpsimd.tensor_tensor(
                    out=tv(s2all, k, 1), in0=bcx(hy, NU).unsqueeze(1),
                    in1=vx1(dxs_t).unsqueeze(1), op=Alu.subtract,
                )

            TRIOS = [(0, 3), (3, 3), (6, 3), (9, 3), (12, 3)]
            POOL_C_TRIOS = {0, 2}
            for k in range(6):
                emit_s2(k)
            near_done = [False]
            for t, (k0, n) in enumerate(TRIOS):
                # masks for the trio (DVE TS, 4x)
                nc.vector.tensor_scalar(
                    out=tv(mkall, k0, n), in0=tv(s2all, k0, n),
                    scalar1=0.0, scalar2=None, op0=Alu.is_ge,
                )
                # p = vgx * gy_u for the trio (DVE TT, 2x)
                nc.vector.tensor_tensor(
                    out=tv(pall, k0, n),
                    in0=vx1(vgx_t).unsqueeze(1).broadcast_to(
                        [P, n, NU, PIT]),
                    in1=gy_t[:, k0 * PIT : (k0 + n) * PIT].rearrange(
                        "p (k i) -> p k i", i=PIT
                    ).unsqueeze(2).broadcast_to([P, n, NU, PIT]),
                    op=Alu.mult,
                )
                ceng = nc.gpsimd if t in POOL_C_TRIOS else nc.vector
                ceng.tensor_tensor(
                    out=tv(call, k0, n), in0=tv(pall, k0, n),
                    in1=tv(mkall, k0, n), op=Alu.mult,
                )
                if t + 2 < len(TRIOS):
                    for k in range(TRIOS[t + 2][0], TRIOS[t + 2][0] + 3):
                        emit_s2(k)
                if t == 1:
                    emit_near_pre()
                    near_done[0] = True
                for k in range(k0, k0 + n):
                    u = U_ORDER[k]
                    mm(call, k, u, wrap=False)
                    if u < 0:
                        mm(call, k, u, wrap=True)
                    if abs(u) <= 4:
                        qt = qq[k % 3]
                        nc.vector.tensor_tensor(
                            out=vx(qt)[:, 4:13, :],
                            in0=vx(nxv_t)[:, 4:13, :],
                            in1=bcx(ny_t[:, k * PIT : (k + 1) * PIT], 9),
                            op=Alu.mult,
                        )
                        mm(qt, 0, u, wrap=False)
                        if u < 0:
                            mm(qt, 0, u, wrap=True)
                    for b in range(8):
                        if completion[b] == k:
                            bank_epilogue(b)
            assert bank_done == bank_total

    nc.compile()
    return nc


def _get_program():
    if "nc" not in _cache:
        _cache["nc"] = _build_program()
        _cache["consts"] = _host_consts()
    return _cache["nc"], _cache["consts"]


def make_in_maps(cif_head):
    _, consts = _get_program()
    in_maps = []
    for c in range(NCORES):
        f0 = c * NF
        shard = np.zeros((NF, 5, HF, WF), np.float32)
        n = max(0, min(F_TOTAL - f0, NF))
        if n > 0:
            shard[:n] = np.asarray(cif_head[f0 : f0 + n], np.float32)
        in_maps.append({"cif": shard, **consts})
    return in_maps


def gather_out(results):
    return np.concatenate(
        [np.asarray(results[c]["out"]) for c in range(NCORES)], axis=0
    )[:F_TOTAL].astype(np.float32)


def kernel(cif_head, caf_head=None, **_unused):
    from concourse.bass_utils import run_bass_kernel_spmd

    nc, _ = _get_program()
    in_maps = make_in_maps(cif_head)
    res = run_bass_kernel_spmd(nc, in_maps, list(range(NCORES))).results
    return gather_out(res)
